# revision 4
# baseline (speedup 1.0000x reference)
"""Neural Factorization Machine — Trainium2 Bass kernel, 8 NeuronCores. v2.

Math (see reference):
    sum_emb = x @ emb; sum_sq = (x*x) @ (emb*emb)
    iv      = 0.5 * (sum_emb^2 - sum_sq)               [B, D]
    h       = relu(iv @ w1.T + b1)                     [B, D]
    inter   = h @ w2.T + b2                            [B, O]
    linear  = x @ lin_w.T + lin_b                      [B, O]
    out     = linear + inter   (computed on host from shipped lin/int)

Sharding (8 cores, core c -> g = c//4 batch half, j = c%4 O-quarter):
  - big linear: rows g*512:(g+1)*512, cols j*1024:(j+1)*1024  (2x4 grid)
  - FM partial sums: core j of each half reduces F-slice j*4096:(j+1)*4096,
    AllReduce(add) over groups [[0..3],[4..7]] completes sum_emb/sum_sq
  - MLP replicated per half (tiny); second layer column-sharded by j.

v2 changes vs baseline (222 us):
  - x^2 and emb^2 are squared ON-CHIP by DVE from the bf16 x / emb tiles
    (statistically identical rounding to host-side fp8 quantization),
    dropping the x2fm (2.1MB) and embj2 (1.05MB) input streams.
  - emb ships as 32*emb so its DVE square is exactly 1024*emb^2 (the fp8
    dynamic-range scale); se PSUM holds 32*se, descaled at evacuation.
  - collective payload, red, out_lin, out_int all bf16; out_sum dropped
    (host adds lin+int in f32).  Total per-core HBM traffic ~36MB vs ~44MB.
  - all input loads ride ONE HWDGE ring (sync/SP) in just-in-time order:
    embt chunks and xf groups interleaved, then wt+xt groups; outputs and
    red on the scalar ring; gpsimd only carries ccin/collective/row-vec
    loads, so SWDGE Q7 latency is off the critical path.
  - iv is computed by DVE as soon as the AllReduce lands (during C).
  - phase A uses 8-ktile DMA groups with se/sq matmuls interleaved
    round-robin (PSUM bank revisit spacing 4): measured 193.7us vs 196.5
    for the 4-ktile/sequential layout; errors bit-identical.

Measured floor context (see memory/kernel history): phase C = 149us of
pure fp8-DoubleRow matmul at the device's sustained ~291ns per N=512
pass (DMA-free/one-LDW/bank-order variants all identical); A ~37us;
B ~7us. Composition is tight against these rates.
"""

import sys

for _p in ("/opt/trn_rl_repo",):
    if _p not in sys.path:
        sys.path.append(_p)

import numpy as np
import ml_dtypes

B, F, D, O = 1024, 16384, 256, 4096
NCORES = 8
GROUPS = [[0, 1, 2, 3], [4, 5, 6, 7]]
Bc, Oc, Fj = B // 2, O // 4, F // 4   # per-core: 512 batch rows, 1024 O cols, 4096 F slice
P = 128
KT_C = F // P      # 128 k-tiles, big linear (paired 2x for DoubleRow)
KT_A = Fj // P     # 32 k-tiles, FM partials
G_C = 8            # k-tiles per DMA group, phase C (4 DoubleRow pairs)
G_A = 4            # k-tiles per DMA group, phase A
WSCALE = 64.0      # host scale on lin_w before fp8 quantization
ZSCALE = 2.0       # host scale on (x - 0.5) before fp8 quantization
PSC = WSCALE * ZSCALE  # PSUM holds PSC * linear; descaled at evacuation
ESCALE = 32.0      # host scale on emb (bf16-exact); emb^2 scale is 1024
E2SCALE = ESCALE * ESCALE

_cache = {}


def _build(repeat=None, phases=("A", "B", "C"), cc_copy=False, b2_zero=False,
           c_dmafree=False, a_dmafree=False, c_xt_eng="sync", c_bufs=3,
           c_one_ldw=False, mm_order="gmn", a_split=False, a_g=None):
    """Emit the SPMD program. repeat=None -> single pass (the real kernel).
    repeat=R wraps the phase body in a hardware loop for timing.
    cc_copy=True replaces the AllReduce with a local DRAM copy (timing-only:
    collectives inside a hardware loop desync the mesh)."""
    import concourse.bass as bass
    import concourse.mybir as mybir
    import concourse.tile as tile
    from concourse import bacc

    f32 = mybir.dt.float32
    f32r = mybir.dt.float32r
    bf16 = mybir.dt.bfloat16
    fp8 = mybir.dt.float8e4
    DR = mybir.MatmulPerfMode.DoubleRow

    nc = bacc.Bacc("TRN2", target_bir_lowering=False)

    zT = nc.dram_tensor("zT", [F, Bc], fp8, kind="ExternalInput")
    xfm = nc.dram_tensor("xfm", [Fj, Bc], bf16, kind="ExternalInput")
    wT = nc.dram_tensor("wT", [F, Oc], fp8, kind="ExternalInput")
    embj = nc.dram_tensor("embj", [Fj, D], bf16, kind="ExternalInput")  # 32*emb
    w1T = nc.dram_tensor("w1T", [D, D], bf16, kind="ExternalInput")
    w2T = nc.dram_tensor("w2T", [D, Oc], bf16, kind="ExternalInput")
    b1 = nc.dram_tensor("b1", [D], f32, kind="ExternalInput")
    b2 = nc.dram_tensor("b2", [Oc], f32, kind="ExternalInput")
    lb = nc.dram_tensor("lb", [Oc], f32, kind="ExternalInput")  # 128*(lin_b+0.5*colsum w)
    out_lin = nc.dram_tensor("out_lin", [Bc, Oc], bf16, kind="ExternalOutput")
    out_int = nc.dram_tensor("out_int", [Bc, Oc], bf16, kind="ExternalOutput")

    zT_t = zT.rearrange("(kt p) b -> p kt b", p=P)
    xfm_t = xfm.rearrange("(kt p) b -> p kt b", p=P)
    wT_t = wT.rearrange("(kt p) o -> p kt o", p=P)
    embj_t = embj.rearrange("(kt p) d -> p kt d", p=P)
    w1T_t = w1T.rearrange("(kt p) d -> p kt d", p=P)
    w2T_t = w2T.rearrange("(kt p) o -> p kt o", p=P)
    out_lin_t = out_lin.rearrange("(mt p) o -> p mt o", p=P)
    out_int_t = out_int.rearrange("(mt p) o -> p mt o", p=P)

    with tile.TileContext(nc) as tc:
        with (
            tc.tile_pool(name="const", bufs=1) as const,
            tc.tile_pool(name="dram", bufs=1, space="DRAM") as dram,
        ):
            ones_f = const.tile([1, P], f32, name="ones_f")
            nc.vector.memset(ones_f[:], 1.0)
            ones = const.tile([1, P], f32r, name="ones")
            nc.vector.tensor_copy(ones[:], ones_f[:])
            b1t = const.tile([P, 2], f32, name="b1t")
            nc.sync.dma_start(out=b1t[:], in_=b1.rearrange("(t p) -> p t", p=P))
            lbrow = const.tile([1, Oc], f32r, name="lbrow")
            nc.gpsimd.dma_start(out=lbrow[:], in_=lb.rearrange("(a o) -> a o", a=1))
            b2row = const.tile([1, Oc], f32r, name="b2row")
            if not b2_zero:
                nc.gpsimd.dma_start(out=b2row[:], in_=b2.rearrange("(a o) -> a o", a=1))
            # persistent SBUF intermediates across phases
            keep = const
            intsb = keep.tile([P, 4, Oc], bf16, name="intsb")
            lin = keep.tile([P, 4, Oc], bf16, name="lin")
            w1s = keep.tile([P, 2, D], bf16, name="w1s")
            w2s = keep.tile([P, 2, Oc], bf16, name="w2s")
            red = keep.tile([P, 4, Bc], bf16, name="red")
            iv = keep.tile([P, 2, Bc], bf16, name="iv")
            t1 = keep.tile([P, 2, Bc], f32, name="t1")
            ccin = dram.tile([4 * P, Bc], bf16, name="ccin")
            ccout = dram.tile([4 * P, Bc], bf16, name="ccout")
            ccin_t = ccin.rearrange("(t p) b -> p t b", p=P)
            ccout_t = ccout.rearrange("(t p) b -> p t b", p=P)

            def phase_A():
                """FM partial sums over this core's F-slice -> ccin, AllReduce.
                sum_emb (se) in bf16 (emb pre-scaled by 32); sum_sq (sq) in
                fp8 DoubleRow with x^2 / (32*emb)^2 squared on-chip by DVE."""
                with (
                    tc.tile_pool(name="emb_pool", bufs=1) as emb_pool,
                    tc.tile_pool(name="xf_pool", bufs=3) as xf_pool,
                    tc.tile_pool(name="x2_pool", bufs=8 if a_split else 3) as x2_pool,
                    tc.tile_pool(name="psA", bufs=1, space="PSUM") as psA,
                    tc.tile_pool(name="evA", bufs=1) as evA,
                ):
                    embt = emb_pool.tile([P, KT_A, D], bf16, name="embt")
                    embt2 = emb_pool.tile([P, KT_A, D], fp8, name="embt2")
                    se = [psA.tile([P, Bc], f32, tag=f"se{mt}", name=f"se{mt}") for mt in range(2)]
                    sq = [psA.tile([P, Bc], f32, tag=f"sq{mt}", name=f"sq{mt}") for mt in range(2)]
                    AG = a_g or G_A
                    NG = KT_A // AG  # xf groups
                    GPC = 8 // AG if AG <= 8 else 1  # groups per emb chunk
                    xfs = []
                    # JIT load order on the sync ring: embt chunk c covers
                    # its AG-sized xf groups; issue it just before them.
                    for c in range(1 if a_dmafree else 4):
                        nc.sync.dma_start(out=embt[:, c * 8:(c + 1) * 8, :],
                                          in_=embj_t[:, c * 8:(c + 1) * 8, :])
                        for kg in range(c * GPC, (c + 1) * GPC):
                            xf = xf_pool.tile([P, AG, Bc], bf16, tag="xf", name="xf")
                            nc.sync.dma_start(out=xf[:], in_=xfm_t[:, kg * AG:(kg + 1) * AG, :])
                            xfs.append(xf)
                        # DVE squares the freshly-landed emb chunk: (32e)^2 = 1024 e^2
                        nc.vector.tensor_mul(embt2[:, c * 8:(c + 1) * 8, :],
                                             embt[:, c * 8:(c + 1) * 8, :],
                                             embt[:, c * 8:(c + 1) * 8, :])
                    x2s = {}
                    sq_defer = []
                    for kg in range(NG):
                        if a_dmafree and kg > 0:
                            xf, x2 = xfs[0], x2s[0]
                        else:
                            xf = xfs[kg]
                            x2 = x2_pool.tile([P, AG, Bc], fp8, tag="x2", name="x2")
                            nc.vector.tensor_mul(x2[:], xf[:], xf[:])
                            x2s[kg] = x2

                        def se_one(g, mt, kg=kg, xf=xf):
                            kt = kg * AG + g
                            st, sp = kt == 0, kt == KT_A - 1
                            ke = kt % 8 if a_dmafree else kt
                            nc.tensor.matmul(se[mt][:], embt[:, ke, mt * P:(mt + 1) * P],
                                             xf[:, g], start=st, stop=sp)

                        def sq_one(g2, mt, kg=kg, x2=x2):
                            kt2 = kg * (AG // 2) + g2
                            st, sp = kt2 == 0, kt2 == KT_A // 2 - 1
                            k2e = (2 * kt2) % 8 if a_dmafree else 2 * kt2
                            gs = slice(2 * g2, 2 * g2 + 2)
                            nc.tensor.matmul(sq[mt][:],
                                             embt2[:, k2e:k2e + 2, mt * P:(mt + 1) * P],
                                             x2[:, gs, :], start=st, stop=sp,
                                             perf_mode=DR)

                        if mm_order == "ilv4":
                            # round-robin se0,se1,sq0,sq1 -> PSUM bank revisit
                            # spacing 4 for the first 2/3 of each group
                            for g2 in range(AG // 2):
                                for mt in range(2):
                                    se_one(2 * g2, mt)
                                for mt in range(2):
                                    sq_one(g2, mt)
                            for g2 in range(AG // 2):
                                for mt in range(2):
                                    se_one(2 * g2 + 1, mt)
                        elif a_split:
                            # se only here; ALL DR sq passes run in one block
                            # after the se stream (one perf-mode switch)
                            for g in range(AG):
                                for mt in range(2):
                                    se_one(g, mt)
                            sq_defer.append(sq_one)
                        elif mm_order == "gmn":
                            for g in range(AG):
                                for mt in range(2):
                                    se_one(g, mt)
                            for g2 in range(AG // 2):
                                for mt in range(2):
                                    sq_one(g2, mt)
                        else:
                            for mt in range(2):
                                for g in range(AG):
                                    se_one(g, mt)
                            for mt in range(2):
                                for g2 in range(AG // 2):
                                    sq_one(g2, mt)
                    for sq_fn in sq_defer:
                        for g2 in range(AG // 2):
                            for mt in range(2):
                                sq_fn(g2, mt)
                    ev = evA.tile([P, 4, Bc], bf16, name="ev")
                    for mt in range(2):
                        nc.vector.tensor_scalar_mul(ev[:, mt, :], se[mt][:], 1.0 / ESCALE)
                        nc.vector.tensor_scalar_mul(ev[:, 2 + mt, :], sq[mt][:], 1.0 / E2SCALE)
                    nc.gpsimd.dma_start(out=ccin_t[:], in_=ev[:])
                if cc_copy:
                    nc.gpsimd.dma_start(out=ccout[:], in_=ccin[:])
                else:
                    nc.gpsimd.collective_compute(
                        "AllReduce", mybir.AluOpType.add, replica_groups=GROUPS,
                        ins=[ccin.opt()], outs=[ccout.opt()],
                    )
                # red + iv as soon as the reduction lands (during phase C;
                # gpsimd/DVE are otherwise idle there)
                nc.gpsimd.dma_start(out=red[:], in_=ccout_t[:])
                se_r, sq_r = red[:, 0:2, :], red[:, 2:4, :]
                nc.vector.tensor_mul(t1[:], se_r, se_r)
                nc.vector.tensor_sub(t1[:], t1[:], sq_r)
                nc.vector.tensor_scalar_mul(iv[:], t1[:], 0.5)

            def phase_B():
                """iv -> h -> interaction_out (+b2) -> intsb, out_int."""
                with (
                    tc.tile_pool(name="mlp", bufs=1) as mlp,
                    tc.tile_pool(name="psB", bufs=2, space="PSUM") as psB,
                ):
                    hsb = mlp.tile([P, 2, Bc], bf16, name="hsb")
                    for mt in range(2):
                        hp = psB.tile([P, Bc], f32, tag="hp", name="hp")
                        for kt in range(2):
                            nc.tensor.matmul(hp[:], w1s[:, kt, mt * P:(mt + 1) * P],
                                             iv[:, kt, :], start=(kt == 0), stop=(kt == 1))
                        nc.scalar.activation(hsb[:, mt, :], hp[:],
                                             mybir.ActivationFunctionType.Relu,
                                             bias=b1t[:, mt:mt + 1])
                    for mb in range(4):
                        for no in range(2):
                            pi = psB.tile([P, 512], f32, tag="pi", name="pi")
                            if not b2_zero:
                                nc.tensor.matmul(pi[:], ones[:],
                                                 b2row[:, no * 512:(no + 1) * 512],
                                                 start=True, stop=False)
                            for kt in range(2):
                                nc.tensor.matmul(pi[:], hsb[:, kt, mb * P:(mb + 1) * P],
                                                 w2s[:, kt, no * 512:(no + 1) * 512],
                                                 start=(b2_zero and kt == 0), stop=(kt == 1))
                            sl = slice(no * 512, (no + 1) * 512)
                            # alternate pi evacuation between DVE and ACT so
                            # neither paces the matmul stream
                            if no == 0:
                                nc.vector.tensor_copy(intsb[:, mb, sl], pi[:])
                            else:
                                nc.scalar.activation(intsb[:, mb, sl], pi[:],
                                                     mybir.ActivationFunctionType.Copy)
                        nc.scalar.dma_start(out=out_int_t[:, mb, :], in_=intsb[:, mb, :])
                    # deferred from phase_C (see note there): last out_lin
                    # halves go on the scalar tail, keeping the sync ring free
                    # for the next iteration's A-input prefetch
                    for m in (2, 3):
                        nc.scalar.dma_start(out=out_lin_t[:, m, :], in_=lin[:, m, :])

            def phase_C(ps23):
                """linear = (zT.T @ wT)/PSC + bias-row ; fp8 DoubleRow."""
                with (
                    tc.tile_pool(name="xt_pool", bufs=c_bufs) as xt_pool,
                    tc.tile_pool(name="wt_pool", bufs=c_bufs) as wt_pool,
                    tc.tile_pool(name="psC", bufs=1, space="PSUM") as psC,
                ):
                    ps01 = [[psC.tile([P, 512], f32, tag=f"ps{m}{n}", name=f"ps{m}{n}")
                             for n in range(2)] for m in range(2)]
                    ps = ps01 + ps23

                    def mm_one(kg, g2, m, n):
                        xt, wt = xwt[kg]
                        k2 = kg * (G_C // 2) + g2
                        gs = slice(2 * g2, 2 * g2 + 2)
                        if c_one_ldw:
                            lhsT = xwt[0][0][:, 0:2, 0:P]
                        else:
                            lhsT = xt[:, gs, m * P:(m + 1) * P]
                        nc.tensor.matmul(ps[m][n][:], lhsT,
                                         wt[:, gs, n * 512:(n + 1) * 512],
                                         start=False, stop=(k2 == KT_C // 2 - 1),
                                         perf_mode=DR)

                    def mm_group(kg, ms):
                        if mm_order == "gmn":
                            # bank changes every matmul
                            for g2 in range(G_C // 2):
                                for m in ms:
                                    for n in range(2):
                                        mm_one(kg, g2, m, n)
                        else:
                            # "mng": 4 consecutive matmuls per PSUM bank
                            # (fewer bank switches; same per-tile k order)
                            for m in ms:
                                for n in range(2):
                                    for g2 in range(G_C // 2):
                                        mm_one(kg, g2, m, n)

                    NKG = KT_C // G_C
                    xt_dma = getattr(nc, c_xt_eng).dma_start
                    xwt = {}
                    for kg in range(NKG):
                        if c_dmafree and kg > 0:
                            xwt[kg] = xwt[0]
                        else:
                            xt = xt_pool.tile([P, G_C, Bc], fp8, tag="xt", name="xt")
                            wt = wt_pool.tile([P, G_C, Oc], fp8, tag="wt", name="wt")
                            xt_dma(out=xt[:], in_=zT_t[:, kg * G_C:(kg + 1) * G_C, :])
                            nc.sync.dma_start(out=wt[:], in_=wT_t[:, kg * G_C:(kg + 1) * G_C, :])
                            xwt[kg] = (xt, wt)
                        if kg == 0:
                            # PE covers the A-psum evacuation latency with the
                            # ps23 half of group 0 before ps01 is initialized.
                            mm_group(0, (2, 3))
                            for m in (0, 1):
                                for n in range(2):
                                    nc.tensor.matmul(ps[m][n][:], ones[:],
                                                     lbrow[:, n * 512:(n + 1) * 512],
                                                     start=True, stop=False)
                            mm_group(0, (0, 1))
                        elif kg < NKG - 2:
                            mm_group(kg, (0, 1, 2, 3))
                    # last two k-groups run m-major so tile (m,·) hits its
                    # stop 2*(3-m) groups early; its evacuation (DVE) and
                    # out_lin DMA then overlap the remaining matmul stream.
                    for m in range(4):
                        for kg in (NKG - 2, NKG - 1):
                            mm_group(kg, (m,))
                        for n in range(2):
                            nc.vector.tensor_scalar_mul(
                                lin[:, m, n * 512:(n + 1) * 512], ps[m][n][:],
                                1.0 / PSC)
                        if m < 2:
                            nc.scalar.dma_start(out=out_lin_t[:, m, :], in_=lin[:, m, :])
                    # mlp weights ride the sync ring after the wt stream.
                    # out_lin m2/m3 must NOT go on sync: they gate on the
                    # late DVE evacuations and would head-of-line-block the
                    # NEXT iteration's A-input prefetch, idling PE >3.4us at
                    # the A boundary (HAM re-throttles -> all of A runs at
                    # 1.2GHz). With B present they move to the scalar tail.
                    nc.sync.dma_start(out=w1s[:], in_=w1T_t[:])
                    nc.sync.dma_start(out=w2s[:], in_=w2T_t[:])
                    if "B" not in phases:
                        for m in (2, 3):
                            nc.sync.dma_start(out=out_lin_t[:, m, :], in_=lin[:, m, :])

            def body():
                # Order: early C-bias inits (PSUM banks disjoint from A's) fill
                # the initial DMA wait; A issues the AllReduce, whose latency
                # hides behind phase C's long matmul stream; B (which consumes
                # the reduced sums) runs after C.
                with tc.tile_pool(name="psCe", bufs=1, space="PSUM") as psCe:
                    ps23 = [[psCe.tile([P, 512], f32, tag=f"ps{m}{n}e", name=f"ps{m}{n}e")
                             for n in range(2)] for m in (2, 3)]
                    if "C" in phases:
                        for mi in range(2):
                            for n in range(2):
                                nc.tensor.matmul(ps23[mi][n][:], ones[:],
                                                 lbrow[:, n * 512:(n + 1) * 512],
                                                 start=True, stop=False)
                    if "A" in phases:
                        phase_A()
                    if "C" in phases:
                        phase_C(ps23)
                    if "B" in phases:
                        phase_B()

            if repeat is None:
                body()
            else:
                import concourse.mybir as _mb
                with tc.For_i(0, repeat, 1, hint_engines=(_mb.EngineType.PE,)) as _i:
                    body()
    _dedupe_ldweights(nc)
    nc.compile()
    return nc


def _dedupe_ldweights(nc):
    """Drop back-to-back InstLdweights that reload the identical stationary
    operand (the legalizer emits one per matmul even when consecutive matmuls
    share weights, e.g. the two n-slices per (k2, m) in phase C). Only
    sync-free duplicates are removed; any PE instruction other than Matmult
    conservatively resets the tracked weight state."""
    for fn in nc.m.functions:
        for blk in fn.blocks:
            last = None
            keep = []
            for inst in blk.instructions:
                nm = type(inst).__name__
                if nm == "InstLdweights":
                    ap = inst.ins[0]
                    key = (ap.memref, ap.offset, tuple(map(tuple, ap.ap)),
                           str(inst.perf_mode), str(inst.tile_position),
                           str(inst.is_transpose))
                    si = inst.sync_info
                    if key == last and (si is None or
                                        (not si.on_wait and not si.on_update)):
                        continue
                    last = key
                elif nm != "InstMatmult":
                    eng = getattr(inst, "engine", None)
                    if eng is not None and str(eng) == "EngineType.PE":
                        last = None
                keep.append(inst)
            if len(keep) != len(blk.instructions):
                blk.instructions.clear()
                blk.instructions.extend(keep)


def _prep_inputs(sae_features, emb, lin_w, lin_b, w1, b1, w2, b2):
    """Host-side shard + transpose + quantize. Returns in_maps for cores 0..7."""
    e4 = ml_dtypes.float8_e4m3
    bf = ml_dtypes.bfloat16
    x = np.asarray(sae_features, dtype=np.float32)
    emb = np.asarray(emb, dtype=np.float32)
    lin_w = np.asarray(lin_w, dtype=np.float32)
    w1T = np.ascontiguousarray(np.asarray(w1, np.float32).T).astype(bf)
    w2 = np.asarray(w2, dtype=np.float32)
    b1 = np.asarray(b1, np.float32)
    b2 = np.asarray(b2, np.float32)
    lin_b = np.asarray(lin_b, np.float32)

    # centered/scaled fp8 operands for the big linear
    z8 = ((x - 0.5) * ZSCALE).astype(e4)               # [B, F] fp8
    w8 = (lin_w * WSCALE).astype(e4)                   # [O, F] fp8
    xb = x.astype(bf)                                  # [B, F] bf16 (FM se path)
    # exact rank-1 correction: linear = z@w + 0.5*colsum(w) + lin_b
    bias_c = PSC * (lin_b + 0.5 * lin_w.sum(axis=1, dtype=np.float64).astype(np.float32))

    zT_half = [np.ascontiguousarray(z8[g * Bc:(g + 1) * Bc, :].T) for g in range(2)]
    xbT_half = [np.ascontiguousarray(xb[g * Bc:(g + 1) * Bc, :].T) for g in range(2)]
    wT_q = [np.ascontiguousarray(w8[j * Oc:(j + 1) * Oc, :].T) for j in range(4)]
    w2T_q = [np.ascontiguousarray(w2[j * Oc:(j + 1) * Oc, :].T).astype(bf) for j in range(4)]
    in_maps = []
    for c in range(NCORES):
        g, j = c // 4, c % 4
        in_maps.append({
            "zT": zT_half[g],
            "xfm": np.ascontiguousarray(xbT_half[g][j * Fj:(j + 1) * Fj, :]),
            "wT": wT_q[j],
            "embj": np.ascontiguousarray(emb[j * Fj:(j + 1) * Fj, :] * ESCALE).astype(bf),
            "w1T": w1T,
            "w2T": w2T_q[j],
            "b1": b1,
            "b2": np.ascontiguousarray(b2[j * Oc:(j + 1) * Oc]),
            "lb": np.ascontiguousarray(bias_c[j * Oc:(j + 1) * Oc]),
        })
    return in_maps


def _gather(results):
    """Assemble full outputs from per-core (g,j) blocks (upcast bf16);
    the summed output is computed host-side in f32."""
    outs = {}
    for key in ("out_lin", "out_int"):
        full = np.empty((B, O), dtype=np.float32)
        for c in range(NCORES):
            g, j = c // 4, c % 4
            full[g * Bc:(g + 1) * Bc, j * Oc:(j + 1) * Oc] = \
                np.asarray(results[c][key]).astype(np.float32)
        outs[key] = full
    out_sum = outs["out_lin"] + outs["out_int"]
    return out_sum, outs["out_lin"], outs["out_int"]


def kernel(sae_features, emb, lin_w, lin_b, w1, b1, w2, b2):
    from concourse.bass_utils import run_bass_kernel_spmd

    b2z = not np.any(np.asarray(b2))
    key = ("nc", b2z)
    if key not in _cache:
        _cache[key] = _build(b2_zero=b2z, a_g=8, mm_order="ilv4")
    nc = _cache[key]
    in_maps = _prep_inputs(sae_features, emb, lin_w, lin_b, w1, b1, w2, b2)
    try:
        res = run_bass_kernel_spmd(nc, in_maps, list(range(NCORES)))
    except Exception:
        # transient device desync/unrecoverable states heal on retry
        import time as _time
        _time.sleep(5)
        res = run_bass_kernel_spmd(nc, in_maps, list(range(NCORES)))
    return _gather(res.results)
